# revision 13
# baseline (speedup 1.0000x reference)
"""Trainium2 Bass kernel for an attention-LSTM decoder.

Strategy (8 NeuronCores, SPMD identical program, per-core inputs differ only
in the vocab-projection shard):
  - The LSTM/attention recurrence over S=20 steps is replicated on every core
    (it is weight-streaming bound; batch-splitting would not shrink it).
  - The vocab projection (V=30000) is sharded 8 ways along V; each core
    computes logits for its 3840-column shard (V padded to 30720).
  - All activations are kept feature-on-partition ("transposed", [feat, B])
    so no per-step transposes are needed anywhere.
  - Attention: scores for all (b, b') pairs are computed as one PE matmul
    (h1^T as stationary, batch duplicated over both 64-partition halves), a
    constant -100 off-block bias is added via an extra K=64 matmul, softmax
    runs on the full row (exact: off-block terms underflow), and the masked
    normalized weight matrix's 128-column blocks directly form the
    block-diagonal operand of the context matmul (two attention positions
    per block via a parity mask).
  - The vocab matmul for step t-1 is issued between the gate matmuls of
    step t so the tensor engine stays busy while ACT/DVE run the softmax
    and LSTM elementwise chains.
"""

import sys
import os
import functools

for _p in ("/opt/trn_rl_repo", os.path.expanduser("~/.axon_site/_ro/trn_rl_repo")):
    if os.path.isdir(_p) and _p not in sys.path:
        sys.path.insert(0, _p)

import numpy as np
import ml_dtypes

import concourse.bacc as bacc
import concourse.mybir as mybir
import concourse.tile as tile
from concourse.bass_utils import run_bass_kernel_spmd

BF16 = ml_dtypes.bfloat16

V, E, H, F = 30000, 256, 512, 2048
B, S = 64, 20
P49 = 49
NC = 8
VP = 30720           # V padded to 8*30*128
VS = VP // NC        # 3840 vocab columns per core
NCBLK = 25           # 25 c-blocks of 128 attention columns
NJ = NCBLK * 128     # 3200 = padded attention column count (p padded 49->50)
NTILE = 400          # scores PSUM n-tile width (8 tiles of 400)
G = 4 * H            # 2048 gates

DT = mybir.dt
AF = mybir.ActivationFunctionType
OP = mybir.AluOpType


def _j_to_bp():
    """attention column order: j = c*128 + par*64 + b <-> (b, p=2c+par)."""
    j = np.arange(NJ)
    c, r = j // 128, j % 128
    par, b = r // 64, r % 64
    return b, 2 * c + par


def build_program(zb0, zb1, zbout, zba):
    """zb*: True when the corresponding bias input is all-zero, letting us
    drop the bias matmuls/tiles for it."""
    nc = bacc.Bacc("TRN2", target_bir_lowering=False, debug=False, num_devices=NC)

    din = {}

    def dram_in(name, shape, dt):
        din[name] = nc.dram_tensor(name, list(shape), dt, kind="ExternalInput")

    dram_in("w0t", [128, 10, G], DT.bfloat16)
    dram_in("w1t", [128, 8, G], DT.bfloat16)
    if not zb0:
        dram_in("bias0", [1, G], DT.bfloat16)
    if not zb1:
        dram_in("bias1", [1, G], DT.bfloat16)
    dram_in("wft", [128, 16, H], DT.bfloat16)
    dram_in("bfp", [128, 4], DT.float32)
    dram_in("wa", [128, 4, H], DT.bfloat16)
    if not zba:
        dram_in("barep", [128, 4, B], DT.bfloat16)
    dram_in("whit", [128, 4, H], DT.bfloat16)
    dram_in("wcit", [128, 4, H], DT.bfloat16)
    dram_in("bhici", [1, 2 * H], DT.bfloat16)
    dram_in("cnnt", [128, 4, B], DT.bfloat16)
    dram_in("featt", [128, 16, NJ], DT.bfloat16)
    dram_in("maskpar", [128, NJ], DT.bfloat16)
    dram_in("maskblk", [64, NJ], DT.bfloat16)
    dram_in("consts", [64, 192], DT.bfloat16)    # idup | ones-row
    dram_in("i128", [128, 128], DT.bfloat16)
    dram_in("xt", [128, 2, S * B], DT.bfloat16)
    dram_in("woutt", [128, 4, VS], DT.bfloat16)
    if not zbout:
        dram_in("bout", [64, VS], DT.bfloat16)

    d_logits = nc.dram_tensor("logits", [S, B, VS], DT.float32, kind="ExternalOutput")
    d_hf = nc.dram_tensor("hfinal", [2, 4, 128, B], DT.float32, kind="ExternalOutput")
    d_cf = nc.dram_tensor("cfinal", [2, 4, 128, B], DT.float32, kind="ExternalOutput")

    with tile.TileContext(nc) as tc:
        _emit(nc, tc, din, d_logits, d_hf, d_cf, zb0, zb1, zbout, zba)
    nc.compile()
    return nc


def _emit(nc, tc, din, d_logits, d_hf, d_cf, zb0, zb1, zbout, zba):
    from contextlib import ExitStack

    with ExitStack() as ctx:
        res = ctx.enter_context(tc.tile_pool(name="res", bufs=1))
        state = ctx.enter_context(tc.tile_pool(name="state", bufs=1))
        work = ctx.enter_context(tc.tile_pool(name="work", bufs=1))
        ps_scores = ctx.enter_context(
            tc.tile_pool(name="ps_scores", bufs=2, space="PSUM"))
        ps_ctx = ctx.enter_context(tc.tile_pool(name="ps_ctx", bufs=1, space="PSUM"))
        ps_gates = ctx.enter_context(
            tc.tile_pool(name="ps_gates", bufs=1, space="PSUM"))
        ps_misc = ctx.enter_context(tc.tile_pool(name="ps_misc", bufs=2, space="PSUM"))

        # ---- persistent state ----
        hcbf = state.tile([128, 4, 192], DT.bfloat16, tag="hcbf")
        h0bf = hcbf[:, :, 0:B]
        h1bf = hcbf[:, :, B:192]        # batch duplicated over both col halves
        c01 = state.tile([128, 4, 128], DT.float32, tag="c01")
        c0 = c01[:, :, 0:B]
        c1 = c01[:, :, B:128]
        cmb = state.tile([128, 7, NJ], DT.bfloat16, tag="cmb")
        projA = cmb[:, 0:4, :]
        maskpar = cmb[:, 4]
        nc.sync.dma_start(maskpar[:], din["maskpar"][:])
        wexp = cmb[:, 5]
        mmat = cmb[0:64, 6]
        projb = state.tile([128, NCBLK, 4, 128], DT.bfloat16, tag="projb")

        consts = res.tile([64, 192], DT.bfloat16, tag="consts")
        nc.sync.dma_start(consts[:], din["consts"][:])
        idup = consts[:, 0:128]
        onesrow = consts[0:1, 128:192]

        # =================== startup ===================
        with (
            tc.tile_pool(name="su", bufs=2) as su,
            tc.tile_pool(name="su1", bufs=1) as su1,
        ):
            wft = su1.tile([128, 16, H], DT.bfloat16, tag="wft")
            nc.sync.dma_start(wft[:], din["wft"][:])
            wa = su1.tile([128, 4, H], DT.bfloat16, tag="wa")
            nc.sync.dma_start(wa[:], din["wa"][:])
            bfp = su1.tile([128, 4], DT.float32, tag="bfp")
            nc.sync.dma_start(bfp[:], din["bfp"][:])
            i128 = su1.tile([128, 128], DT.bfloat16, tag="i128")
            nc.sync.dma_start(i128[:], din["i128"][:])
            projT = su1.tile([128, 4, NJ], DT.bfloat16, tag="projT")

            # ---- h/c init from cnn_feat ----
            cnnt = su1.tile([128, 4, B], DT.bfloat16, tag="cnnt")
            nc.sync.dma_start(cnnt[:], din["cnnt"][:])
            bhici = su1.tile([1, 2 * H], DT.bfloat16, tag="bhici")
            nc.sync.dma_start(bhici[:], din["bhici"][:])
            for idx, wname in enumerate(("whit", "wcit")):
                wsb = su.tile([128, 4, H], DT.bfloat16, tag="w_init")
                nc.sync.dma_start(wsb[:], din[wname][:])
                brow = bhici[:, idx * H:(idx + 1) * H]
                ps = ps_scores.tile([128, 4, B], DT.float32, tag="sc")
                for hc in range(4):
                    nc.tensor.matmul(
                        ps[:, hc, :], brow[:, 128 * hc:128 * (hc + 1)], onesrow[:],
                        start=True, stop=False)
                    for kc in range(4):
                        nc.tensor.matmul(
                            ps[:, hc, :],
                            wsb[:, kc, 128 * hc:128 * (hc + 1)],
                            cnnt[:, kc, :],
                            start=False, stop=(kc == 3))
                if idx == 0:
                    nc.scalar.activation(h0bf[:], ps[:], AF.Tanh)
                    nc.scalar.activation(h1bf[:, :, 0:B], ps[:], AF.Tanh)
                    nc.scalar.activation(h1bf[:, :, B:128], ps[:], AF.Tanh)
                else:
                    nc.scalar.activation(c0[:], ps[:], AF.Tanh)
                    nc.scalar.activation(c1[:], ps[:], AF.Tanh)

            # ---- proj^T = Wf^T-contraction of feat^T (+bf), in j-order ----
            FSL = 200
            for sl in range(NJ // FSL):
                ft = su.tile([128, 16, FSL], DT.bfloat16, tag="featsl")
                nc.sync.dma_start(ft[:], din["featt"][:, :, sl * FSL:(sl + 1) * FSL])
                for hc in range(4):
                    ps = ps_scores.tile([128, FSL], DT.float32, tag="sc")
                    for kc in range(16):
                        nc.tensor.matmul(
                            ps[:], wft[:, kc, 128 * hc:128 * (hc + 1)],
                            ft[:, kc, :],
                            start=(kc == 0), stop=(kc == 15))
                    nc.vector.tensor_scalar_add(
                        projT[:, hc, sl * FSL:(sl + 1) * FSL], ps[:],
                        bfp[:, hc:hc + 1])

            # ---- projA^T = Wa-contraction of proj^T ----
            SL = 400
            for sl in range(NJ // SL):
                for hc in range(4):
                    ps = ps_scores.tile([128, SL], DT.float32, tag="sc")
                    for kc in range(4):
                        nc.tensor.matmul(
                            ps[:], wa[:, kc, 128 * hc:128 * (hc + 1)],
                            projT[:, kc, sl * SL:(sl + 1) * SL],
                            start=(kc == 0), stop=(kc == 3))
                    nc.vector.tensor_copy(projA[:, hc, sl * SL:(sl + 1) * SL], ps[:])

            # ---- mmat: -100 off-block, + (ba . proj) on-block ----
            nc.sync.dma_start(mmat[:], din["maskblk"][:])
            if zba:
                # mmat = 100*onblock - 100
                nc.vector.tensor_scalar(mmat[:], mmat[:], 100.0, -100.0,
                                        OP.mult, OP.add)
            else:
                barep = su1.tile([128, 4, B], DT.bfloat16, tag="barep")
                nc.sync.dma_start(barep[:], din["barep"][:])
                for sl in range(NJ // SL):
                    ps = ps_scores.tile([64, SL], DT.float32, tag="sc")
                    for kc in range(4):
                        nc.tensor.matmul(
                            ps[:], barep[:, kc, :],
                            projT[:, kc, sl * SL:(sl + 1) * SL],
                            start=(kc == 0), stop=(kc == 3))
                    s = slice(sl * SL, (sl + 1) * SL)
                    # mmat = (sbias + 100)*onblock - 100
                    nc.vector.scalar_tensor_tensor(
                        mmat[:, s], ps[:], 100.0, mmat[:, s], OP.add, OP.mult)
                    nc.vector.tensor_scalar_add(mmat[:, s], mmat[:, s], -100.0)

            # ---- proj_b2: transpose projT c-blocks -> [q, (c, hc, h)] ----
            for c in range(NCBLK):
                for hc in range(4):
                    pst = ps_scores.tile([128, 128], DT.bfloat16, tag="sc")
                    nc.tensor.transpose(
                        pst[:], projT[:, hc, 128 * c:128 * (c + 1)], i128[:])
                    nc.vector.tensor_copy(projb[:, c, hc, :], pst[:])

        # ---- step-phase residents: their pool opens after the startup
        # pools close, so the allocator can reuse the startup zone ----
        big = ctx.enter_context(tc.tile_pool(name="big", bufs=1))

        def load(name, shape, dt):
            t = big.tile(shape, dt, tag=name)
            nc.sync.dma_start(t[:], din[name][:])
            return t

        w0t = load("w0t", [128, 10, G], DT.bfloat16)
        w1t = load("w1t", [128, 8, G], DT.bfloat16)
        bias0 = None if zb0 else load("bias0", [1, G], DT.bfloat16)
        bias1 = None if zb1 else load("bias1", [1, G], DT.bfloat16)
        xt = load("xt", [128, 2, S * B], DT.bfloat16)
        woutt = load("woutt", [128, 4, VS], DT.bfloat16)
        bout = None if zbout else load("bout", [64, VS], DT.bfloat16)

        # =================== steps ===================
        NSC = NJ // NTILE  # 8 scores n-tiles

        def vocab_half(tv, half):
            nv = VS // 4
            for q in (2 * half, 2 * half + 1):
                base = q * nv
                lsbv = work.tile([64, nv], DT.float32, tag="lsb")
                for n0 in range(0, nv, 480):
                    ps = ps_misc.tile([64, 480], DT.float32, tag="voc")
                    for kc in range(4):
                        nc.tensor.matmul(
                            ps[:], h1bf[:, kc, 0:B],
                            woutt[:, kc, base + n0:base + n0 + 480],
                            start=(kc == 0), stop=(kc == 3))
                    dst = lsbv[:, n0:n0 + 480]
                    if zbout:
                        nc.vector.tensor_copy(dst, ps[:])
                    else:
                        nc.vector.scalar_tensor_tensor(
                            dst, ps[:], 1.0,
                            bout[:, base + n0:base + n0 + 480], OP.mult, OP.add)
                nc.sync.dma_start(d_logits[tv][:, base:base + nv], lsbv[:])

        for t in range(S):
            # ---- scores + softmax over masked rows ----
            sm = work.tile([128, 16], DT.float32, tag="sm")
            for nt in range(NSC):
                ps = ps_scores.tile([128, NTILE], DT.float32, tag="sc")
                s = slice(nt * NTILE, (nt + 1) * NTILE)
                nc.tensor.matmul(ps[:], idup[:], mmat[:, s], start=True, stop=False)
                for kc in range(4):
                    nc.tensor.matmul(
                        ps[:], h1bf[:, kc, :], projA[:, kc, s],
                        start=False, stop=(kc == 3))
                nc.scalar.activation(
                    wexp[:, s], ps[:], AF.Exp, accum_out=sm[:, nt:nt + 1])
            nc.vector.tensor_reduce(
                sm[:, 8:9], sm[:, 0:NSC], mybir.AxisListType.X, OP.add)
            nc.vector.reciprocal(sm[:, 9:10], sm[:, 8:9])
            wmat = wexp
            nc.vector.scalar_tensor_tensor(
                wmat[:], wexp[:], sm[:, 9:10], maskpar[:], OP.mult, OP.mult)

            # ---- context (transposed), two p per 128-block via parity ----
            psc = ps_ctx.tile([128, 4, 128], DT.float32, tag="ctx")
            for hc in range(4):
                for c in range(NCBLK):
                    nc.tensor.matmul(
                        psc[:, hc, :], projb[:, c, hc, :],
                        wmat[:, 128 * c:128 * (c + 1)],
                        start=(c == 0), stop=(c == NCBLK - 1))
            wpk = work.tile([128, 20, B], DT.bfloat16, tag="wpk")
            ctxbf = wpk[:, 0:4, :]
            tmp0 = wpk[:, 4:8, :]
            tanh0 = wpk[:, 8:12, :]
            tmp1 = wpk[:, 12:16, :]
            tanh1 = wpk[:, 16:20, :]
            ctxf = work.tile([128, 4, 128], DT.bfloat16, tag="ctxf")
            nc.vector.tensor_copy(ctxf[:], psc[:])
            nc.vector.tensor_tensor(
                ctxbf[:], ctxf[:, :, 0:B], ctxf[:, :, B:128], OP.add)

            # ---- layer-0 gates ----
            pg0 = ps_gates.tile([128, 16, B], DT.float32, tag="g")
            for gc in range(16):
                gs = slice(128 * gc, 128 * (gc + 1))
                first = True
                if not zb0:
                    nc.tensor.matmul(pg0[:, gc, :], bias0[:, gs], onesrow[:],
                                     start=True, stop=False)
                    first = False
                for kc in range(2):
                    nc.tensor.matmul(
                        pg0[:, gc, :], w0t[:, kc, gs],
                        xt[:, kc, t * B:(t + 1) * B], start=first, stop=False)
                    first = False
                for kc in range(4):
                    nc.tensor.matmul(
                        pg0[:, gc, :], w0t[:, 2 + kc, gs], ctxbf[:, kc, :],
                        start=False, stop=False)
                for kc in range(4):
                    nc.tensor.matmul(
                        pg0[:, gc, :], w0t[:, 6 + kc, gs], h0bf[:, kc, :],
                        start=False, stop=(kc == 3))

            act0 = work.tile([128, 16, B], DT.bfloat16, tag="act0")
            nc.scalar.activation(act0[:, 0:8, :], pg0[:, 0:8, :], AF.Sigmoid)
            nc.scalar.activation(act0[:, 8:12, :], pg0[:, 8:12, :], AF.Tanh)
            nc.scalar.activation(act0[:, 12:16, :], pg0[:, 12:16, :], AF.Sigmoid)

            if t > 0:
                vocab_half(t - 1, 0)

            # ---- layer-0 elementwise ----
            nc.vector.tensor_tensor(
                tmp0[:], act0[:, 0:4, :], act0[:, 8:12, :], OP.mult)
            nc.vector.tensor_tensor(c0[:], act0[:, 4:8, :], c0[:], OP.mult)
            nc.vector.tensor_tensor(c0[:], c0[:], tmp0[:], OP.add)
            nc.scalar.activation(tanh0[:], c0[:], AF.Tanh)
            nc.vector.tensor_tensor(h0bf[:], act0[:, 12:16, :], tanh0[:], OP.mult)

            # ---- layer-1 gates ----
            pg1 = ps_gates.tile([128, 16, B], DT.float32, tag="g")
            for gc in range(16):
                gs = slice(128 * gc, 128 * (gc + 1))
                first = True
                if not zb1:
                    nc.tensor.matmul(pg1[:, gc, :], bias1[:, gs], onesrow[:],
                                     start=True, stop=False)
                    first = False
                for kc in range(4):
                    nc.tensor.matmul(
                        pg1[:, gc, :], w1t[:, kc, gs], h0bf[:, kc, :],
                        start=first, stop=False)
                    first = False
                for kc in range(4):
                    nc.tensor.matmul(
                        pg1[:, gc, :], w1t[:, 4 + kc, gs], h1bf[:, kc, 0:B],
                        start=False, stop=(kc == 3))

            act1 = work.tile([128, 16, B], DT.bfloat16, tag="act1")
            nc.scalar.activation(act1[:, 0:8, :], pg1[:, 0:8, :], AF.Sigmoid)
            nc.scalar.activation(act1[:, 8:12, :], pg1[:, 8:12, :], AF.Tanh)
            nc.scalar.activation(act1[:, 12:16, :], pg1[:, 12:16, :], AF.Sigmoid)

            if t > 0:
                vocab_half(t - 1, 1)

            # ---- layer-1 elementwise ----
            nc.vector.tensor_tensor(
                tmp1[:], act1[:, 0:4, :], act1[:, 8:12, :], OP.mult)
            nc.vector.tensor_tensor(c1[:], act1[:, 4:8, :], c1[:], OP.mult)
            nc.vector.tensor_tensor(c1[:], c1[:], tmp1[:], OP.add)
            nc.scalar.activation(tanh1[:], c1[:], AF.Tanh)
            nc.vector.tensor_tensor(
                h1bf[:, :, 0:B], act1[:, 12:16, :], tanh1[:], OP.mult)
            nc.vector.tensor_tensor(
                h1bf[:, :, B:128], act1[:, 12:16, :], tanh1[:], OP.mult)

            if t == S - 1:
                hff = work.tile([128, 8, B], DT.float32, tag="lsb")
                nc.vector.tensor_tensor(
                    hff[:, 0:4, :], act0[:, 12:16, :], tanh0[:], OP.mult)
                nc.vector.tensor_tensor(
                    hff[:, 4:8, :], act1[:, 12:16, :], tanh1[:], OP.mult)
                for l in range(2):
                    nc.sync.dma_start(
                        d_hf[l].rearrange("a p b -> p a b"),
                        hff[:, 4 * l:4 * (l + 1), :])
                for l, tl in ((0, c0), (1, c1)):
                    nc.sync.dma_start(
                        d_cf[l].rearrange("a p b -> p a b"), tl[:])

        # tail vocab for the last step
        vocab_half(S - 1, 0)
        vocab_half(S - 1, 1)


# ---------------------------------------------------------------------------
# host-side input prep
# ---------------------------------------------------------------------------
def _prep(inputs):
    f32 = lambda a: np.asarray(a, dtype=np.float32)
    bf = lambda a: np.ascontiguousarray(np.asarray(a, dtype=np.float32).astype(BF16))

    word_ids = np.asarray(inputs["word_ids"]).astype(np.int64)
    emb = f32(inputs["embed_table"])
    Wf, bfv = f32(inputs["Wf"]), f32(inputs["bf"])
    Wa, ba = f32(inputs["Wa"]), f32(inputs["ba"])
    W_ih0, W_hh0 = f32(inputs["W_ih0"]), f32(inputs["W_hh0"])
    b_ih0, b_hh0 = f32(inputs["b_ih0"]), f32(inputs["b_hh0"])
    W_ih1, W_hh1 = f32(inputs["W_ih1"]), f32(inputs["W_hh1"])
    b_ih1, b_hh1 = f32(inputs["b_ih1"]), f32(inputs["b_hh1"])
    W_hi, b_hi = f32(inputs["W_hi"]), f32(inputs["b_hi"])
    W_ci, b_ci = f32(inputs["W_ci"]), f32(inputs["b_ci"])
    W_out, b_out = f32(inputs["W_out"]), f32(inputs["b_out"])
    cnn = f32(inputs["cnn_feat"])
    feat = f32(inputs["attn_enhanced_feat"]).reshape(B, F, P49)

    bias0 = b_ih0 + b_hh0
    bias1 = b_ih1 + b_hh1
    flags = (not bias0.any(), not bias1.any(), not b_out.any(), not ba.any())

    shared = {}
    w0 = np.concatenate([W_ih0, W_hh0], axis=1).T        # [1280, 2048]
    shared["w0t"] = bf(w0.reshape(10, 128, G).transpose(1, 0, 2))
    w1 = np.concatenate([W_ih1, W_hh1], axis=1).T        # [1024, 2048]
    shared["w1t"] = bf(w1.reshape(8, 128, G).transpose(1, 0, 2))
    if not flags[0]:
        shared["bias0"] = bf(bias0[None, :])
    if not flags[1]:
        shared["bias1"] = bf(bias1[None, :])
    shared["wft"] = bf(Wf.T.reshape(16, 128, H).transpose(1, 0, 2))
    shared["bfp"] = np.ascontiguousarray(bfv.reshape(4, 128).T)
    shared["wa"] = bf(Wa.reshape(4, 128, H).transpose(1, 0, 2))
    if not flags[3]:
        shared["barep"] = bf(
            np.repeat(ba.reshape(4, 128, 1), B, axis=2).transpose(1, 0, 2))
    shared["whit"] = bf(W_hi.T.reshape(4, 128, H).transpose(1, 0, 2))
    shared["wcit"] = bf(W_ci.T.reshape(4, 128, H).transpose(1, 0, 2))
    shared["bhici"] = bf(np.concatenate([b_hi, b_ci])[None, :])
    shared["cnnt"] = bf(cnn.T.reshape(4, 128, B).transpose(1, 0, 2))

    consts = np.zeros((64, 192), dtype=np.float32)
    consts[:, 0:64] = np.eye(64)
    consts[:, 64:128] = np.eye(64)
    consts[0, 128:192] = 1.0
    shared["consts"] = bf(consts)
    shared["i128"] = bf(np.eye(128))

    jb, jp = _j_to_bp()
    featt = np.zeros((F, NJ), dtype=np.float32)
    valid = jp < P49
    featt[:, valid] = feat[jb[valid], :, jp[valid]].T
    shared["featt"] = bf(featt.reshape(16, 128, NJ).transpose(1, 0, 2))

    rows = np.arange(64)
    onblk = (rows[:, None] == jb[None, :]) & (jp < P49)[None, :]
    shared["maskblk"] = bf(onblk.astype(np.float32))
    q = np.arange(128)
    mp = ((q[:, None] % 64) == jb[None, :]) \
        & ((q[:, None] // 64) == (jp % 2)[None, :]) & (jp < P49)[None, :]
    shared["maskpar"] = bf(mp.astype(np.float32))

    ids = word_ids.T.reshape(-1)                          # s-major
    xemb = emb[ids] * (ids != 0)[:, None]                 # [S*B, 256]
    shared["xt"] = bf(xemb.T.reshape(2, 128, S * B).transpose(1, 0, 2))

    wout_pad = np.zeros((VP, H), dtype=np.float32)
    wout_pad[:V] = W_out
    bout_pad = np.zeros((VP,), dtype=np.float32)
    bout_pad[:V] = b_out
    in_maps = []
    for c in range(NC):
        m = dict(shared)
        sl = slice(c * VS, (c + 1) * VS)
        m["woutt"] = bf(wout_pad[sl].T.reshape(4, 128, VS).transpose(1, 0, 2))
        if not flags[2]:
            m["bout"] = bf(np.broadcast_to(bout_pad[sl], (64, VS)))
        in_maps.append(m)
    return in_maps, flags


@functools.lru_cache(maxsize=4)
def _compiled(flags):
    return build_program(*flags)


def kernel(**inputs):
    in_maps, flags = _prep(inputs)
    nc = _compiled(flags)
    res = run_bass_kernel_spmd(nc, in_maps, list(range(NC)))
    logits = np.concatenate(
        [res.results[c]["logits"] for c in range(NC)], axis=2)  # [S, B, VP]
    outputs = np.ascontiguousarray(logits[:, :, :V].transpose(1, 0, 2))
    hf = res.results[0]["hfinal"].reshape(2, H, B).transpose(0, 2, 1)
    cf = res.results[0]["cfinal"].reshape(2, H, B).transpose(0, 2, 1)
    return outputs, np.ascontiguousarray(hf), np.ascontiguousarray(cf)
